# revision 17
# baseline (speedup 1.0000x reference)
"""AttLayer pooling kernel for TRN2, 8 NeuronCores, data-parallel over batch.

Reference computation (per batch b):
    z   = x @ W + b            # [T, D]
    th  = tanh(z)              # [T, D]
    dot = th @ uw              # [T]
    a~  = exp(dot) * mask      # [T]
    out = (x^T @ a~) / (sum(a~) + EPS)   # [D]

Shapes: x [64, 2048, 256] f32, W [256, 256], b/uw [256], mask [64, 2048] i32.
Each core handles 8 batches = 16384 rows = 128 row-tiles of [128, 256].

Key layout facts:
  - TensorE matmul contracts over the partition dim of both operands, so the
    z matmul needs x^T (d on partitions). x^T is produced on-chip by PE
    transposes (identity matmul) in bf16 and copied PSUM->SBUF by DVE.
  - The whole einsum1 path runs in bf16 (x is cast f32->bf16 during the DMA
    load by SWDGE). einsum2 accumulates in f32 PSUM.
  - Normalization is linear, so a~ is used unnormalized for the weighted sum
    and the division by (sum + EPS) happens once at the end.
"""

import os
from contextlib import ExitStack

import ml_dtypes
import numpy as np

import concourse.bass as bass
import concourse.mybir as mybir
import concourse.tile as tile
from concourse import bacc
from concourse.bass_utils import run_bass_kernel_spmd

B, T, D = 64, 2048, 256
N_CORES = 8
B_LOC = B // N_CORES          # 8 batches per core
RT = B_LOC * T // 128         # 128 row-tiles per core
NG = RT // 4                  # 32 groups of 4 row-tiles
# x DMA chunk sizes in row-tiles: small starter chunks so PE can begin early,
# then 1MB (8-tile) chunks for bandwidth. Each chunk is one processing block.
CHUNKS = [4, 4] + [8] * 15
assert sum(CHUNKS) == RT
CHUNK_START = [sum(CHUNKS[:i]) for i in range(len(CHUNKS))]
EPS = 1e-7
WARMUP_MMS = 28               # dummy transposes to lift PE HAM to 2.4GHz
JPB = RT // B_LOC             # 16 row-tiles per batch
# chunk index whose completion finishes batch b: chunks [4,4,8...] ->
# b0 done after chunk 2, then every 2nd chunk
B_DONE_CHUNK = {2 + 2 * i: i for i in range(B_LOC)}

F32 = mybir.dt.float32
BF16 = mybir.dt.bfloat16

_CACHE = {}

LAST_RESULT = None  # BassKernelResults of the most recent run (for test.py)


def _build_graph(bias_zero=True):
    nc = bacc.Bacc(
        "TRN2", target_bir_lowering=False, debug=False, num_devices=N_CORES
    )

    x = nc.dram_tensor("x", [RT, 128, D], F32, kind="ExternalInput").ap()
    w = nc.dram_tensor("w", [2, 128, D], BF16, kind="ExternalInput").ap()
    uw = nc.dram_tensor("uw", [2, 128], BF16, kind="ExternalInput").ap()
    bb = nc.dram_tensor("bb", [2, 128], F32, kind="ExternalInput").ap()
    maskt = nc.dram_tensor("maskt", [128, RT], F32, kind="ExternalInput").ap()
    ident = nc.dram_tensor("ident", [128, 128], BF16, kind="ExternalInput").ap()
    ones = nc.dram_tensor("ones", [128, 1], BF16, kind="ExternalInput").ap()
    onesf = nc.dram_tensor("onesf", [128, 1], F32, kind="ExternalInput").ap()
    out_d = nc.dram_tensor("out", [1, B_LOC * D], F32, kind="ExternalOutput").ap()

    with tile.TileContext(nc) as tc, ExitStack() as ctx:
        constp = ctx.enter_context(tc.tile_pool(name="const", bufs=1))
        xbp = ctx.enter_context(tc.tile_pool(name="xb", bufs=1))
        xtp = ctx.enter_context(tc.tile_pool(name="xt", bufs=2))
        thp = ctx.enter_context(tc.tile_pool(name="th", bufs=3))
        accp = ctx.enter_context(tc.tile_pool(name="acc", bufs=3))
        miscp = ctx.enter_context(tc.tile_pool(name="misc", bufs=1))
        ptp = ctx.enter_context(tc.tile_pool(name="pt", bufs=2, space="PSUM"))
        pzp = ctx.enter_context(tc.tile_pool(name="pz", bufs=3, space="PSUM"))
        pdp = ctx.enter_context(tc.tile_pool(name="pd", bufs=1, space="PSUM"))
        psp = ctx.enter_context(tc.tile_pool(name="ps", bufs=2, space="PSUM"))

        # ---- constants. ident first: it gates the PE warm-up. HWDGE triggers
        # issue serially (~0.7us each) so order = criticality. ----
        id_sb = constp.tile([128, 128], BF16)
        nc.sync.dma_start(out=id_sb[:], in_=ident)
        w_sb = constp.tile([128, 2, D], BF16)
        nc.sync.dma_start(out=w_sb[:], in_=w.transpose([1, 0, 2]))
        uw_sb = constp.tile([128, 2], BF16)
        nc.sync.dma_start(out=uw_sb[:], in_=uw.transpose([1, 0]))
        b_sb = constp.tile([128, 2], F32)
        nc.sync.dma_start(out=b_sb[:], in_=bb.transpose([1, 0]))
        maskt_sb = constp.tile([128, RT], F32)
        nc.sync.dma_start(out=maskt_sb[:], in_=maskt)
        ones_sb = constp.tile([128, 1], BF16)
        nc.sync.dma_start(out=ones_sb[:], in_=ones)
        onesf_sb = constp.tile([128, 1], F32)
        nc.sync.dma_start(out=onesf_sb[:], in_=onesf)

        # ---- x load: f32 -> bf16 cast during SWDGE DMA ----
        xch = []
        for c, (cs, c0) in enumerate(zip(CHUNKS, CHUNK_START)):
            t = xbp.tile([128, cs, D], BF16, tag=f"xb{c}")
            nc.gpsimd.dma_start(out=t[:], in_=x[c0 : c0 + cs].transpose([1, 0, 2]))
            xch.append(t)

        def xb(j):  # [128 t', 256 d] bf16 for row-tile j
            for c in range(len(CHUNKS) - 1, -1, -1):
                if j >= CHUNK_START[c]:
                    return xch[c][:, j - CHUNK_START[c], :]
            raise AssertionError

        # ---- PE warm-up: dummy transposes while the first x chunks stream in.
        # Back-to-back MMs give HAM its ~3.4us of sustained activity so the
        # real matmul stream runs at 2.4GHz from the start. ----
        wu = ptp.tile([128, 1024], BF16, tag="pt")
        for i in range(WARMUP_MMS):
            nc.tensor.transpose(
                wu[:, (i % 8) * 128 : (i % 8) * 128 + 128], id_sb[:], id_sb[:]
            )

        # persistent-ish state
        dot_ps = pdp.tile([128, RT], F32)   # column j = uw-dots for row-tile j
        e_sb = miscp.tile([128, RT], F32)   # exp(dot)
        af32 = miscp.tile([128, RT], F32)   # exp(dot) * mask, f32 (DVE path + sums)
        a16 = miscp.tile([128, RT], BF16)   # exp(dot) * mask, bf16 (PE path)
        rcp = miscp.tile([1, B_LOC], F32)   # 1 / (sum_t a~ + eps), per batch
        out_sb = miscp.tile([1, B_LOC * D], F32)
        accs = {}                           # DVE batches: b -> [128, D] bf16
        pos = {}                            # PE batches: b -> [1, D] f32 psum

        def b_on_pe(b):
            return b % 2 == 0

        # ---- single fused phase: one processing block per x chunk ----
        for c, (nt, t0) in enumerate(zip(CHUNKS, CHUNK_START)):
            wdt = nt * 128
            xt = xtp.tile([128, 2, wdt], BF16, tag="xt")
            for k in range(2):
                pt = ptp.tile([128, wdt], BF16, tag="pt")
                for r in range(nt):
                    nc.tensor.transpose(
                        pt[:, r * 128 : (r + 1) * 128],
                        xb(t0 + r)[:, k * 128 : (k + 1) * 128],
                        id_sb[:],
                    )
                nc.vector.tensor_copy(out=xt[:, k, :], in_=pt[:])

            # z^T = W.T x^T in 512-column slabs, tanh per (slab, e-half)
            for h in range(nt // 4):
                ths = []
                for m in range(2):
                    zz = pzp.tile([128, 512], F32, tag="pz")
                    for k in range(2):
                        nc.tensor.matmul(
                            zz[:],
                            w_sb[:, k, m * 128 : (m + 1) * 128],
                            xt[:, k, h * 512 : (h + 1) * 512],
                            start=(k == 0),
                            stop=(k == 1),
                        )
                    th = thp.tile([128, 512], BF16, tag="th")
                    nc.scalar.activation(
                        th[:],
                        zz[:],
                        mybir.ActivationFunctionType.Tanh,
                        bias=b_sb[:, m : m + 1],
                    )
                    ths.append(th)
                for r in range(4):
                    j = t0 + 4 * h + r
                    for m in range(2):
                        nc.tensor.matmul(
                            dot_ps[:, j : j + 1],
                            ths[m][:, r * 128 : (r + 1) * 128],
                            uw_sb[:, m : m + 1],
                            start=(m == 0),
                            stop=(m == 1),
                        )

            # exp + mask for this chunk's dot columns
            bc = t0 // JPB  # batch this chunk belongs to
            nc.scalar.activation(
                e_sb[:, t0 : t0 + nt],
                dot_ps[:, t0 : t0 + nt],
                mybir.ActivationFunctionType.Exp,
            )
            nc.vector.tensor_mul(
                af32[:, t0 : t0 + nt],
                e_sb[:, t0 : t0 + nt],
                maskt_sb[:, t0 : t0 + nt],
            )
            if b_on_pe(bc):
                nc.vector.tensor_mul(
                    a16[:, t0 : t0 + nt],
                    e_sb[:, t0 : t0 + nt],
                    maskt_sb[:, t0 : t0 + nt],
                )

            # weighted sum over t: PE matmul for even batches, DVE fused
            # multiply-add for odd batches
            for r in range(nt):
                j = t0 + r
                b = j // JPB
                if b_on_pe(b):
                    if b not in pos:
                        po_t = psp.tile([1, D], F32, tag="po")
                        pos[b] = po_t
                    nc.tensor.matmul(
                        pos[b][:],
                        a16[:, j : j + 1],
                        xb(j)[:],
                        start=(j % JPB == 0),
                        stop=(j % JPB == JPB - 1),
                    )
                elif b not in accs:
                    acc_t = accp.tile([128, D], BF16, tag="acc")
                    accs[b] = acc_t
                    nc.vector.tensor_scalar(
                        out=accs[b][:],
                        in0=xb(j)[:],
                        scalar1=af32[:, j : j + 1],
                        scalar2=None,
                        op0=mybir.AluOpType.mult,
                    )
                else:
                    nc.vector.scalar_tensor_tensor(
                        out=accs[b][:],
                        in0=xb(j)[:],
                        scalar=af32[:, j : j + 1],
                        in1=accs[b][:],
                        op0=mybir.AluOpType.mult,
                        op1=mybir.AluOpType.add,
                    )

            # batches completed by this chunk: sums, normalize, store
            if c in B_DONE_CHUNK:
                b = B_DONE_CHUNK[c]
                s_ps = psp.tile([1, 16], F32, tag="po")
                nc.tensor.matmul(
                    s_ps[:],
                    onesf_sb[:],
                    af32[:, b * JPB : (b + 1) * JPB],
                    start=True,
                    stop=True,
                )
                sb1 = miscp.tile([1, 1], F32, tag="sb1")
                nc.vector.reduce_sum(sb1[:], s_ps[:], axis=mybir.AxisListType.X)
                nc.vector.tensor_scalar_add(sb1[:], sb1[:], EPS)
                nc.vector.reciprocal(rcp[:, b : b + 1], sb1[:])

                if b_on_pe(b):
                    po = pos.pop(b)
                else:
                    po = psp.tile([1, D], F32, tag="po")
                    nc.tensor.matmul(
                        po[:], ones_sb[:], accs[b][:], start=True, stop=True
                    )
                nc.vector.tensor_scalar(
                    out=out_sb[:, b * D : (b + 1) * D],
                    in0=po[:],
                    scalar1=rcp[:, b : b + 1],
                    scalar2=None,
                    op0=mybir.AluOpType.mult,
                )
                nc.sync.dma_start(
                    out=out_d[:, b * D : (b + 1) * D],
                    in_=out_sb[:, b * D : (b + 1) * D],
                )

    nc.compile()
    return nc


def _get_graph(bias_zero=True):
    key = ("nc", bias_zero)
    if key not in _CACHE:
        _CACHE[key] = _build_graph(bias_zero)
    return _CACHE[key]


def kernel(x, W, b, uw, mask):
    global LAST_RESULT
    b = np.asarray(b, dtype=np.float32)
    nc = _get_graph(bias_zero=not np.any(b))

    x = np.asarray(x, dtype=np.float32)
    W = np.asarray(W, dtype=np.float32)
    uw = np.asarray(uw, dtype=np.float32)
    mask_f = np.asarray(mask).astype(np.float32)

    wb = np.ascontiguousarray(W.reshape(2, 128, D).astype(ml_dtypes.bfloat16))
    uwb = np.ascontiguousarray(uw.reshape(2, 128).astype(ml_dtypes.bfloat16))
    bf = np.ascontiguousarray(b.reshape(2, 128).astype(np.float32))
    ident = np.eye(128, dtype=ml_dtypes.bfloat16)
    ones = np.ones((128, 1), dtype=ml_dtypes.bfloat16)
    onesf = np.ones((128, 1), dtype=np.float32)

    in_maps = []
    for c in range(N_CORES):
        xs = np.ascontiguousarray(
            x[c * B_LOC : (c + 1) * B_LOC].reshape(RT, 128, D)
        )
        ms = mask_f[c * B_LOC : (c + 1) * B_LOC]  # [8, 2048]
        maskt = np.ascontiguousarray(
            ms.reshape(B_LOC, RT // B_LOC, 128).transpose(2, 0, 1).reshape(128, RT)
        )
        in_maps.append(
            {
                "x": xs,
                "w": wb,
                "uw": uwb,
                "bb": bf,
                "maskt": maskt,
                "ident": ident,
                "ones": ones,
                "onesf": onesf,
            }
        )

    res = run_bass_kernel_spmd(
        nc,
        in_maps,
        core_ids=list(range(N_CORES)),
        trace=bool(int(os.environ.get("ATT_TRACE", "0"))),
    )
    LAST_RESULT = res
    outs = [
        np.asarray(res.results[c]["out"]).reshape(B_LOC, D)
        for c in range(N_CORES)
    ]
    return np.concatenate(outs, axis=0)


# revision 18
# speedup vs baseline: 1.1556x; 1.1556x over previous
"""AttLayer pooling kernel for TRN2, 8 NeuronCores, data-parallel over batch.

Reference computation (per batch b):
    z   = x @ W + b            # [T, D]
    th  = tanh(z)              # [T, D]
    dot = th @ uw              # [T]
    a~  = exp(dot) * mask      # [T]
    out = (x^T @ a~) / (sum(a~) + EPS)   # [D]

Shapes: x [64, 2048, 256] f32, W [256, 256], b/uw [256], mask [64, 2048] i32.
Each core handles 8 batches = 16384 rows = 128 row-tiles of [128, 256].

Key layout facts:
  - TensorE matmul contracts over the partition dim of both operands, so the
    z matmul needs x^T (d on partitions). x^T is produced on-chip by PE
    transposes (identity matmul) in bf16 and copied PSUM->SBUF by DVE.
  - The whole einsum1 path runs in bf16 (x is cast f32->bf16 during the DMA
    load by SWDGE). einsum2 accumulates in f32 PSUM.
  - Normalization is linear, so a~ is used unnormalized for the weighted sum
    and the division by (sum + EPS) happens once at the end.
"""

import os
from contextlib import ExitStack

import ml_dtypes
import numpy as np

import concourse.bass as bass
import concourse.mybir as mybir
import concourse.tile as tile
from concourse import bacc
from concourse.bass_utils import run_bass_kernel_spmd

B, T, D = 64, 2048, 256
N_CORES = 8
B_LOC = B // N_CORES          # 8 batches per core
RT = B_LOC * T // 128         # 128 row-tiles per core
NG = RT // 4                  # 32 groups of 4 row-tiles
# x DMA chunk sizes in row-tiles: small starter chunks so PE can begin early,
# then 1MB (8-tile) chunks for bandwidth. Each chunk is one processing block.
CHUNKS = [4, 4] + [8] * 15
assert sum(CHUNKS) == RT
CHUNK_START = [sum(CHUNKS[:i]) for i in range(len(CHUNKS))]
EPS = 1e-7
WARMUP_MMS = 28               # dummy transposes to lift PE HAM to 2.4GHz
JPB = RT // B_LOC             # 16 row-tiles per batch
# chunk index whose completion finishes batch b: chunks [4,4,8...] ->
# b0 done after chunk 2, then every 2nd chunk
B_DONE_CHUNK = {2 + 2 * i: i for i in range(B_LOC)}

F32 = mybir.dt.float32
BF16 = mybir.dt.bfloat16

_CACHE = {}

LAST_RESULT = None  # BassKernelResults of the most recent run (for test.py)


def _build_graph(bias_zero=True):
    nc = bacc.Bacc(
        "TRN2", target_bir_lowering=False, debug=False, num_devices=N_CORES
    )

    x = nc.dram_tensor("x", [RT, 128, D], F32, kind="ExternalInput").ap()
    w = nc.dram_tensor("w", [2, 128, D], BF16, kind="ExternalInput").ap()
    uw = nc.dram_tensor("uw", [2, 128], BF16, kind="ExternalInput").ap()
    bb = nc.dram_tensor("bb", [2, 128], F32, kind="ExternalInput").ap()
    maskt = nc.dram_tensor("maskt", [128, RT], F32, kind="ExternalInput").ap()
    ident = nc.dram_tensor("ident", [128, 128], BF16, kind="ExternalInput").ap()
    ones = nc.dram_tensor("ones", [128, 1], BF16, kind="ExternalInput").ap()
    onesf = nc.dram_tensor("onesf", [128, 1], F32, kind="ExternalInput").ap()
    out_d = nc.dram_tensor("out", [1, B_LOC * D], F32, kind="ExternalOutput").ap()

    with tile.TileContext(nc) as tc, ExitStack() as ctx:
        constp = ctx.enter_context(tc.tile_pool(name="const", bufs=1))
        xbp = ctx.enter_context(tc.tile_pool(name="xb", bufs=1))
        xtp = ctx.enter_context(tc.tile_pool(name="xt", bufs=2))
        thp = ctx.enter_context(tc.tile_pool(name="th", bufs=3))
        accp = ctx.enter_context(tc.tile_pool(name="acc", bufs=3))
        miscp = ctx.enter_context(tc.tile_pool(name="misc", bufs=1))
        ptp = ctx.enter_context(tc.tile_pool(name="pt", bufs=2, space="PSUM"))
        pzp = ctx.enter_context(tc.tile_pool(name="pz", bufs=2, space="PSUM"))
        pdp = ctx.enter_context(tc.tile_pool(name="pd", bufs=2, space="PSUM"))
        psp = ctx.enter_context(tc.tile_pool(name="ps", bufs=2, space="PSUM"))

        # ---- constants. ident first: it gates the PE warm-up. HWDGE triggers
        # issue serially (~0.7us each) so order = criticality. ----
        id_sb = constp.tile([128, 128], BF16)
        nc.sync.dma_start(out=id_sb[:], in_=ident)
        w_sb = constp.tile([128, 2, D], BF16)
        nc.sync.dma_start(out=w_sb[:], in_=w.transpose([1, 0, 2]))
        uw_sb = constp.tile([128, 2], BF16)
        nc.sync.dma_start(out=uw_sb[:], in_=uw.transpose([1, 0]))
        b_sb = constp.tile([128, 2], F32)
        nc.sync.dma_start(out=b_sb[:], in_=bb.transpose([1, 0]))
        maskt_sb = constp.tile([128, RT], F32)
        nc.sync.dma_start(out=maskt_sb[:], in_=maskt)
        ones_sb = constp.tile([128, 1], BF16)
        nc.sync.dma_start(out=ones_sb[:], in_=ones)
        onesf_sb = constp.tile([128, 1], F32)
        nc.sync.dma_start(out=onesf_sb[:], in_=onesf)

        # ---- x load: f32 -> bf16 cast during SWDGE DMA ----
        xch = []
        for c, (cs, c0) in enumerate(zip(CHUNKS, CHUNK_START)):
            t = xbp.tile([128, cs, D], BF16, tag=f"xb{c}")
            nc.gpsimd.dma_start(out=t[:], in_=x[c0 : c0 + cs].transpose([1, 0, 2]))
            xch.append(t)

        def xb(j):  # [128 t', 256 d] bf16 for row-tile j
            for c in range(len(CHUNKS) - 1, -1, -1):
                if j >= CHUNK_START[c]:
                    return xch[c][:, j - CHUNK_START[c], :]
            raise AssertionError

        # ---- PE warm-up: dummy transposes while the first x chunks stream in.
        # Back-to-back MMs give HAM its ~3.4us of sustained activity so the
        # real matmul stream runs at 2.4GHz from the start. ----
        wu = ptp.tile([128, 1024], BF16, tag="pt")
        for i in range(WARMUP_MMS):
            nc.tensor.transpose(
                wu[:, (i % 8) * 128 : (i % 8) * 128 + 128], id_sb[:], id_sb[:]
            )

        # persistent-ish state
        e_sb = miscp.tile([128, RT], F32)   # exp(dot)
        af32 = miscp.tile([128, RT], F32)   # exp(dot) * mask, f32 (DVE path + sums)
        a16 = miscp.tile([128, RT], BF16)   # exp(dot) * mask, bf16 (PE path)
        rcp = miscp.tile([1, B_LOC], F32)   # 1 / (sum_t a~ + eps), per batch
        out_sb = miscp.tile([1, B_LOC * D], F32)
        accs = {}                           # DVE batches: b -> [128, D] bf16
        pos = {}                            # PE batches: b -> [1, D] f32 psum

        def b_on_pe(b):
            return b % 2 == 0

        # ---- single fused phase: one processing block per x chunk ----
        for c, (nt, t0) in enumerate(zip(CHUNKS, CHUNK_START)):
            wdt = nt * 128
            dot_ps = pdp.tile([128, nt], F32, tag="dot")  # this chunk's dots
            xt = xtp.tile([128, 2, wdt], BF16, tag="xt")
            for k in range(2):
                pt = ptp.tile([128, wdt], BF16, tag="pt")
                for r in range(nt):
                    nc.tensor.transpose(
                        pt[:, r * 128 : (r + 1) * 128],
                        xb(t0 + r)[:, k * 128 : (k + 1) * 128],
                        id_sb[:],
                    )
                nc.vector.tensor_copy(out=xt[:, k, :], in_=pt[:])

            # z^T = W.T x^T in 512-column slabs, tanh per (slab, e-half)
            for h in range(nt // 4):
                ths = []
                for m in range(2):
                    zz = pzp.tile([128, 512], F32, tag="pz")
                    for k in range(2):
                        nc.tensor.matmul(
                            zz[:],
                            w_sb[:, k, m * 128 : (m + 1) * 128],
                            xt[:, k, h * 512 : (h + 1) * 512],
                            start=(k == 0),
                            stop=(k == 1),
                        )
                    th = thp.tile([128, 512], BF16, tag="th")
                    nc.scalar.activation(
                        th[:],
                        zz[:],
                        mybir.ActivationFunctionType.Tanh,
                        bias=b_sb[:, m : m + 1],
                    )
                    ths.append(th)
                for r in range(4):
                    j = t0 + 4 * h + r
                    for m in range(2):
                        nc.tensor.matmul(
                            dot_ps[:, 4 * h + r : 4 * h + r + 1],
                            ths[m][:, r * 128 : (r + 1) * 128],
                            uw_sb[:, m : m + 1],
                            start=(m == 0),
                            stop=(m == 1),
                        )

            # exp + mask for this chunk's dot columns
            bc = t0 // JPB  # batch this chunk belongs to
            nc.scalar.activation(
                e_sb[:, t0 : t0 + nt],
                dot_ps[:],
                mybir.ActivationFunctionType.Exp,
            )
            nc.vector.tensor_mul(
                af32[:, t0 : t0 + nt],
                e_sb[:, t0 : t0 + nt],
                maskt_sb[:, t0 : t0 + nt],
            )
            if b_on_pe(bc):
                nc.vector.tensor_mul(
                    a16[:, t0 : t0 + nt],
                    e_sb[:, t0 : t0 + nt],
                    maskt_sb[:, t0 : t0 + nt],
                )

            # weighted sum over t: PE matmul for even batches, DVE fused
            # multiply-add for odd batches
            for r in range(nt):
                j = t0 + r
                b = j // JPB
                if b_on_pe(b):
                    if b not in pos:
                        po_t = psp.tile([1, D], F32, tag="po")
                        pos[b] = po_t
                    nc.tensor.matmul(
                        pos[b][:],
                        a16[:, j : j + 1],
                        xb(j)[:],
                        start=(j % JPB == 0),
                        stop=(j % JPB == JPB - 1),
                    )
                elif b not in accs:
                    acc_t = accp.tile([128, D], BF16, tag="acc")
                    accs[b] = acc_t
                    nc.vector.tensor_scalar(
                        out=accs[b][:],
                        in0=xb(j)[:],
                        scalar1=af32[:, j : j + 1],
                        scalar2=None,
                        op0=mybir.AluOpType.mult,
                    )
                else:
                    nc.vector.scalar_tensor_tensor(
                        out=accs[b][:],
                        in0=xb(j)[:],
                        scalar=af32[:, j : j + 1],
                        in1=accs[b][:],
                        op0=mybir.AluOpType.mult,
                        op1=mybir.AluOpType.add,
                    )

            # batches completed by this chunk: sums, normalize, store
            if c in B_DONE_CHUNK:
                b = B_DONE_CHUNK[c]
                s_ps = psp.tile([1, 16], F32, tag="po")
                nc.tensor.matmul(
                    s_ps[:],
                    onesf_sb[:],
                    af32[:, b * JPB : (b + 1) * JPB],
                    start=True,
                    stop=True,
                )
                sb1 = miscp.tile([1, 1], F32, tag="sb1")
                nc.vector.reduce_sum(sb1[:], s_ps[:], axis=mybir.AxisListType.X)
                nc.vector.tensor_scalar_add(sb1[:], sb1[:], EPS)
                nc.vector.reciprocal(rcp[:, b : b + 1], sb1[:])

                if b_on_pe(b):
                    po = pos.pop(b)
                else:
                    po = psp.tile([1, D], F32, tag="po")
                    nc.tensor.matmul(
                        po[:], ones_sb[:], accs[b][:], start=True, stop=True
                    )
                nc.vector.tensor_scalar(
                    out=out_sb[:, b * D : (b + 1) * D],
                    in0=po[:],
                    scalar1=rcp[:, b : b + 1],
                    scalar2=None,
                    op0=mybir.AluOpType.mult,
                )
                nc.sync.dma_start(
                    out=out_d[:, b * D : (b + 1) * D],
                    in_=out_sb[:, b * D : (b + 1) * D],
                )

    nc.compile()
    return nc


def _get_graph(bias_zero=True):
    key = ("nc", bias_zero)
    if key not in _CACHE:
        _CACHE[key] = _build_graph(bias_zero)
    return _CACHE[key]


def kernel(x, W, b, uw, mask):
    global LAST_RESULT
    b = np.asarray(b, dtype=np.float32)
    nc = _get_graph(bias_zero=not np.any(b))

    x = np.asarray(x, dtype=np.float32)
    W = np.asarray(W, dtype=np.float32)
    uw = np.asarray(uw, dtype=np.float32)
    mask_f = np.asarray(mask).astype(np.float32)

    wb = np.ascontiguousarray(W.reshape(2, 128, D).astype(ml_dtypes.bfloat16))
    uwb = np.ascontiguousarray(uw.reshape(2, 128).astype(ml_dtypes.bfloat16))
    bf = np.ascontiguousarray(b.reshape(2, 128).astype(np.float32))
    ident = np.eye(128, dtype=ml_dtypes.bfloat16)
    ones = np.ones((128, 1), dtype=ml_dtypes.bfloat16)
    onesf = np.ones((128, 1), dtype=np.float32)

    in_maps = []
    for c in range(N_CORES):
        xs = np.ascontiguousarray(
            x[c * B_LOC : (c + 1) * B_LOC].reshape(RT, 128, D)
        )
        ms = mask_f[c * B_LOC : (c + 1) * B_LOC]  # [8, 2048]
        maskt = np.ascontiguousarray(
            ms.reshape(B_LOC, RT // B_LOC, 128).transpose(2, 0, 1).reshape(128, RT)
        )
        in_maps.append(
            {
                "x": xs,
                "w": wb,
                "uw": uwb,
                "bb": bf,
                "maskt": maskt,
                "ident": ident,
                "ones": ones,
                "onesf": onesf,
            }
        )

    res = run_bass_kernel_spmd(
        nc,
        in_maps,
        core_ids=list(range(N_CORES)),
        trace=bool(int(os.environ.get("ATT_TRACE", "0"))),
    )
    LAST_RESULT = res
    outs = [
        np.asarray(res.results[c]["out"]).reshape(B_LOC, D)
        for c in range(N_CORES)
    ]
    return np.concatenate(outs, axis=0)


# revision 19
# speedup vs baseline: 1.1715x; 1.0138x over previous
"""AttLayer pooling kernel for TRN2, 8 NeuronCores, data-parallel over batch.

Reference computation (per batch b):
    z   = x @ W + b            # [T, D]
    th  = tanh(z)              # [T, D]
    dot = th @ uw              # [T]
    a~  = exp(dot) * mask      # [T]
    out = (x^T @ a~) / (sum(a~) + EPS)   # [D]

Shapes: x [64, 2048, 256] f32, W [256, 256], b/uw [256], mask [64, 2048] i32.
Each core handles 8 batches = 16384 rows = 128 row-tiles of [128, 256].

Key layout facts:
  - TensorE matmul contracts over the partition dim of both operands, so the
    z matmul needs x^T (d on partitions). x^T is produced on-chip by PE
    transposes (identity matmul) in bf16 and copied PSUM->SBUF by DVE.
  - The whole einsum1 path runs in bf16 (x is cast f32->bf16 during the DMA
    load by SWDGE). einsum2 accumulates in f32 PSUM.
  - Normalization is linear, so a~ is used unnormalized for the weighted sum
    and the division by (sum + EPS) happens once at the end.
"""

import os
from contextlib import ExitStack

import ml_dtypes
import numpy as np

import concourse.bass as bass
import concourse.mybir as mybir
import concourse.tile as tile
from concourse import bacc
from concourse.bass_utils import run_bass_kernel_spmd

B, T, D = 64, 2048, 256
N_CORES = 8
B_LOC = B // N_CORES          # 8 batches per core
RT = B_LOC * T // 128         # 128 row-tiles per core
NG = RT // 4                  # 32 groups of 4 row-tiles
# x DMA chunk sizes in row-tiles: small starter chunks so PE can begin early,
# then 1MB (8-tile) chunks for bandwidth. Each chunk is one processing block.
CHUNKS = [4, 4] + [8] * 15
assert sum(CHUNKS) == RT
CHUNK_START = [sum(CHUNKS[:i]) for i in range(len(CHUNKS))]
EPS = 1e-7
WARMUP_MMS = 28               # dummy transposes to lift PE HAM to 2.4GHz
JPB = RT // B_LOC             # 16 row-tiles per batch
# chunk index whose completion finishes batch b: chunks [4,4,8...] ->
# b0 done after chunk 2, then every 2nd chunk
B_DONE_CHUNK = {2 + 2 * i: i for i in range(B_LOC)}

F32 = mybir.dt.float32
BF16 = mybir.dt.bfloat16

_CACHE = {}

LAST_RESULT = None  # BassKernelResults of the most recent run (for test.py)


def _build_graph(bias_zero=True):
    nc = bacc.Bacc(
        "TRN2", target_bir_lowering=False, debug=False, num_devices=N_CORES
    )

    x = nc.dram_tensor("x", [RT, 128, D], F32, kind="ExternalInput").ap()
    w = nc.dram_tensor("w", [2, 128, D], BF16, kind="ExternalInput").ap()
    uw = nc.dram_tensor("uw", [2, 128], BF16, kind="ExternalInput").ap()
    bb = nc.dram_tensor("bb", [2, 128], F32, kind="ExternalInput").ap()
    maskt = nc.dram_tensor("maskt", [128, RT], F32, kind="ExternalInput").ap()
    ident = nc.dram_tensor("ident", [128, 128], BF16, kind="ExternalInput").ap()
    ones = nc.dram_tensor("ones", [128, 1], BF16, kind="ExternalInput").ap()
    onesf = nc.dram_tensor("onesf", [128, 1], F32, kind="ExternalInput").ap()
    out_d = nc.dram_tensor("out", [1, B_LOC * D], F32, kind="ExternalOutput").ap()

    with tile.TileContext(nc) as tc, ExitStack() as ctx:
        constp = ctx.enter_context(tc.tile_pool(name="const", bufs=1))
        xbp = ctx.enter_context(tc.tile_pool(name="xb", bufs=1))
        xtp = ctx.enter_context(tc.tile_pool(name="xt", bufs=2))
        thp = ctx.enter_context(tc.tile_pool(name="th", bufs=3))
        accp = ctx.enter_context(tc.tile_pool(name="acc", bufs=3))
        miscp = ctx.enter_context(tc.tile_pool(name="misc", bufs=1))
        ptp = ctx.enter_context(tc.tile_pool(name="pt", bufs=2, space="PSUM"))
        pzp = ctx.enter_context(tc.tile_pool(name="pz", bufs=2, space="PSUM"))
        pdp = ctx.enter_context(tc.tile_pool(name="pd", bufs=2, space="PSUM"))
        psp = ctx.enter_context(tc.tile_pool(name="ps", bufs=2, space="PSUM"))

        # ---- constants. ident first: it gates the PE warm-up. HWDGE triggers
        # issue serially (~0.7us each) so order = criticality. ----
        id_sb = constp.tile([128, 128], BF16)
        nc.sync.dma_start(out=id_sb[:], in_=ident)
        w_sb = constp.tile([128, 2, D], BF16)
        nc.sync.dma_start(out=w_sb[:], in_=w.transpose([1, 0, 2]))
        uw_sb = constp.tile([128, 2], BF16)
        nc.sync.dma_start(out=uw_sb[:], in_=uw.transpose([1, 0]))
        b_sb = constp.tile([128, 2], F32)
        nc.sync.dma_start(out=b_sb[:], in_=bb.transpose([1, 0]))
        maskt_sb = constp.tile([128, RT], F32)
        nc.sync.dma_start(out=maskt_sb[:], in_=maskt)
        ones_sb = constp.tile([128, 1], BF16)
        nc.sync.dma_start(out=ones_sb[:], in_=ones)
        onesf_sb = constp.tile([128, 1], F32)
        nc.sync.dma_start(out=onesf_sb[:], in_=onesf)

        # ---- x load: f32 -> bf16 cast during SWDGE DMA ----
        xch = []
        for c, (cs, c0) in enumerate(zip(CHUNKS, CHUNK_START)):
            t = xbp.tile([128, cs, D], BF16, tag=f"xb{c}")
            nc.gpsimd.dma_start(out=t[:], in_=x[c0 : c0 + cs].transpose([1, 0, 2]))
            xch.append(t)

        def xb(j):  # [128 t', 256 d] bf16 for row-tile j
            for c in range(len(CHUNKS) - 1, -1, -1):
                if j >= CHUNK_START[c]:
                    return xch[c][:, j - CHUNK_START[c], :]
            raise AssertionError

        # ---- engine warm-up while the first x chunks stream in: prime the
        # ScalarE activation tables (one tiny Exp -> ACT_TABLE_LOAD fires
        # early), and run dummy PE transposes back-to-back so HAM reaches
        # 2.4GHz before the real matmul stream starts. The warm-up psum lives
        # in the dot pool so chunk 0's transposes get a pt slot immediately. ----
        prim = miscp.tile([1, 1], F32)
        nc.scalar.activation(
            prim[:],
            id_sb[0:1, 0:1],
            mybir.ActivationFunctionType.Exp,
        )
        wu = pdp.tile([128, 128], BF16, tag="dot")
        for i in range(WARMUP_MMS):
            nc.tensor.transpose(wu[:], id_sb[:], id_sb[:])

        # persistent-ish state
        e_sb = miscp.tile([128, RT], F32)   # exp(dot)
        af32 = miscp.tile([128, RT], F32)   # exp(dot) * mask, f32 (DVE path + sums)
        a16 = miscp.tile([128, RT], BF16)   # exp(dot) * mask, bf16 (PE path)
        rcp = miscp.tile([1, B_LOC], F32)   # 1 / (sum_t a~ + eps), per batch
        out_sb = miscp.tile([1, B_LOC * D], F32)
        accs = {}                           # DVE batches: b -> [128, D] bf16
        pos = {}                            # PE batches: b -> [1, D] f32 psum

        def b_on_pe(b):
            return b % 2 == 0

        # ---- single fused phase: one processing block per x chunk ----
        for c, (nt, t0) in enumerate(zip(CHUNKS, CHUNK_START)):
            wdt = nt * 128
            dot_ps = pdp.tile([128, nt], F32, tag="dot")  # this chunk's dots
            xt = xtp.tile([128, 2, wdt], BF16, tag="xt")
            for k in range(2):
                pt = ptp.tile([128, wdt], BF16, tag="pt")
                for r in range(nt):
                    nc.tensor.transpose(
                        pt[:, r * 128 : (r + 1) * 128],
                        xb(t0 + r)[:, k * 128 : (k + 1) * 128],
                        id_sb[:],
                    )
                nc.vector.tensor_copy(out=xt[:, k, :], in_=pt[:])

            # z^T = W.T x^T in 512-column slabs, tanh per (slab, e-half)
            for h in range(nt // 4):
                ths = []
                for m in range(2):
                    zz = pzp.tile([128, 512], F32, tag="pz")
                    for k in range(2):
                        nc.tensor.matmul(
                            zz[:],
                            w_sb[:, k, m * 128 : (m + 1) * 128],
                            xt[:, k, h * 512 : (h + 1) * 512],
                            start=(k == 0),
                            stop=(k == 1),
                        )
                    th = thp.tile([128, 512], BF16, tag="th")
                    nc.scalar.activation(
                        th[:],
                        zz[:],
                        mybir.ActivationFunctionType.Tanh,
                        bias=b_sb[:, m : m + 1],
                    )
                    ths.append(th)
                for r in range(4):
                    j = t0 + 4 * h + r
                    for m in range(2):
                        nc.tensor.matmul(
                            dot_ps[:, 4 * h + r : 4 * h + r + 1],
                            ths[m][:, r * 128 : (r + 1) * 128],
                            uw_sb[:, m : m + 1],
                            start=(m == 0),
                            stop=(m == 1),
                        )

            # exp + mask for this chunk's dot columns
            bc = t0 // JPB  # batch this chunk belongs to
            nc.scalar.activation(
                e_sb[:, t0 : t0 + nt],
                dot_ps[:],
                mybir.ActivationFunctionType.Exp,
            )
            nc.vector.tensor_mul(
                af32[:, t0 : t0 + nt],
                e_sb[:, t0 : t0 + nt],
                maskt_sb[:, t0 : t0 + nt],
            )
            if b_on_pe(bc):
                nc.vector.tensor_mul(
                    a16[:, t0 : t0 + nt],
                    e_sb[:, t0 : t0 + nt],
                    maskt_sb[:, t0 : t0 + nt],
                )

            # weighted sum over t: PE matmul for even batches, DVE fused
            # multiply-add for odd batches
            for r in range(nt):
                j = t0 + r
                b = j // JPB
                if b_on_pe(b):
                    if b not in pos:
                        po_t = psp.tile([1, D], F32, tag="po")
                        pos[b] = po_t
                    nc.tensor.matmul(
                        pos[b][:],
                        a16[:, j : j + 1],
                        xb(j)[:],
                        start=(j % JPB == 0),
                        stop=(j % JPB == JPB - 1),
                    )
                elif b not in accs:
                    acc_t = accp.tile([128, D], BF16, tag="acc")
                    accs[b] = acc_t
                    nc.vector.tensor_scalar(
                        out=accs[b][:],
                        in0=xb(j)[:],
                        scalar1=af32[:, j : j + 1],
                        scalar2=None,
                        op0=mybir.AluOpType.mult,
                    )
                else:
                    nc.vector.scalar_tensor_tensor(
                        out=accs[b][:],
                        in0=xb(j)[:],
                        scalar=af32[:, j : j + 1],
                        in1=accs[b][:],
                        op0=mybir.AluOpType.mult,
                        op1=mybir.AluOpType.add,
                    )

            # batches completed by this chunk: sums, normalize, store
            if c in B_DONE_CHUNK:
                b = B_DONE_CHUNK[c]
                s_ps = psp.tile([1, 16], F32, tag="po")
                nc.tensor.matmul(
                    s_ps[:],
                    onesf_sb[:],
                    af32[:, b * JPB : (b + 1) * JPB],
                    start=True,
                    stop=True,
                )
                sb1 = miscp.tile([1, 1], F32, tag="sb1")
                nc.vector.reduce_sum(sb1[:], s_ps[:], axis=mybir.AxisListType.X)
                nc.vector.tensor_scalar_add(sb1[:], sb1[:], EPS)
                nc.vector.reciprocal(rcp[:, b : b + 1], sb1[:])

                if b_on_pe(b):
                    po = pos.pop(b)
                else:
                    po = psp.tile([1, D], F32, tag="po")
                    nc.tensor.matmul(
                        po[:], ones_sb[:], accs[b][:], start=True, stop=True
                    )
                nc.vector.tensor_scalar(
                    out=out_sb[:, b * D : (b + 1) * D],
                    in0=po[:],
                    scalar1=rcp[:, b : b + 1],
                    scalar2=None,
                    op0=mybir.AluOpType.mult,
                )
                nc.sync.dma_start(
                    out=out_d[:, b * D : (b + 1) * D],
                    in_=out_sb[:, b * D : (b + 1) * D],
                )

    nc.compile()
    return nc


def _get_graph(bias_zero=True):
    key = ("nc", bias_zero)
    if key not in _CACHE:
        _CACHE[key] = _build_graph(bias_zero)
    return _CACHE[key]


def kernel(x, W, b, uw, mask):
    global LAST_RESULT
    b = np.asarray(b, dtype=np.float32)
    nc = _get_graph(bias_zero=not np.any(b))

    x = np.asarray(x, dtype=np.float32)
    W = np.asarray(W, dtype=np.float32)
    uw = np.asarray(uw, dtype=np.float32)
    mask_f = np.asarray(mask).astype(np.float32)

    wb = np.ascontiguousarray(W.reshape(2, 128, D).astype(ml_dtypes.bfloat16))
    uwb = np.ascontiguousarray(uw.reshape(2, 128).astype(ml_dtypes.bfloat16))
    bf = np.ascontiguousarray(b.reshape(2, 128).astype(np.float32))
    ident = np.eye(128, dtype=ml_dtypes.bfloat16)
    ones = np.ones((128, 1), dtype=ml_dtypes.bfloat16)
    onesf = np.ones((128, 1), dtype=np.float32)

    in_maps = []
    for c in range(N_CORES):
        xs = np.ascontiguousarray(
            x[c * B_LOC : (c + 1) * B_LOC].reshape(RT, 128, D)
        )
        ms = mask_f[c * B_LOC : (c + 1) * B_LOC]  # [8, 2048]
        maskt = np.ascontiguousarray(
            ms.reshape(B_LOC, RT // B_LOC, 128).transpose(2, 0, 1).reshape(128, RT)
        )
        in_maps.append(
            {
                "x": xs,
                "w": wb,
                "uw": uwb,
                "bb": bf,
                "maskt": maskt,
                "ident": ident,
                "ones": ones,
                "onesf": onesf,
            }
        )

    res = run_bass_kernel_spmd(
        nc,
        in_maps,
        core_ids=list(range(N_CORES)),
        trace=bool(int(os.environ.get("ATT_TRACE", "0"))),
    )
    LAST_RESULT = res
    outs = [
        np.asarray(res.results[c]["out"]).reshape(B_LOC, D)
        for c in range(N_CORES)
    ]
    return np.concatenate(outs, axis=0)
